# revision 1
# baseline (speedup 1.0000x reference)
"""DMGatedGCNConv (3-hop gated GCN) on 8 Trainium2 NeuronCores.

Strategy: shard target nodes across 8 cores; each core computes the full
q|v projection tables (replicated compute, no collectives), gathers per-edge
q/v/k rows + one-hot scatter rows via SWDGE dma_gather, applies the gated
message elementwise (DVE/ACT), and scatter-reduces messages into per-bin
PSUM tiles with TensorE matmuls (one-hot weighted). Skip connection + biases
are folded in via fp32 matmuls and host-side weight preprocessing.
"""
import os
import sys

sys.path.insert(0, "/opt/trn_rl_repo")

import numpy as np
import ml_dtypes

from concourse import bacc, mybir
from concourse.tile import TileContext
from concourse import bass_utils

BF16 = ml_dtypes.bfloat16

# problem geometry (hardcoded per harness contract)
CFG = dict(
    N=50000,   # nodes
    C=128,     # channels
    P=3,       # hops
    NCORES=8,
    NB=49,     # bins (chunks of <=128 target nodes) per core
    SPLIT=24960,  # q|v table split so gather indices fit int16
    SLAB=2048,    # node-phase xT slab width
)

LAST_EXEC_NS = [None]  # test harness reads this after kernel()


def _wrap_idx(ids):
    """int16 index array -> [128, L/16] layout dma_gather expects
    (slot i lives at [i % 16, i // 16], replicated for the 8 Q7 cores)."""
    ids = np.asarray(ids, np.int16)
    assert len(ids) % 16 == 0
    return np.tile(np.ascontiguousarray(ids.reshape(-1, 16).T), (8, 1))


def _softmax(d, axis=0):
    e = np.exp(d - d.max(axis=axis, keepdims=True))
    return e / e.sum(axis=axis, keepdims=True)


def _preprocess(x, eis, ews, Wk, bk, Wq, bq, Wv, bv, Wskip, cbias, d, hop_bias, cfg):
    N, C, P = cfg["N"], cfg["C"], cfg["P"]
    M, NB, SPLIT = cfg["NCORES"], cfg["NB"], cfg["SPLIT"]
    NS = N // M            # nodes per core shard
    NPAD = NB * 128        # padded shard size

    x = np.asarray(x, np.float32)
    dw = _softmax(np.asarray(d, np.float32), axis=0)          # [P, C]

    # fold d_weight into v / skip / cbias; fold hop_bias once
    WqvT, WkT, WskipT = [], [], []
    qv_bias, k_bias = [], []
    cbias_sum = np.zeros(C, np.float32)
    for p in range(P):
        wq = np.asarray(Wq[p], np.float32)
        wv = np.asarray(Wv[p], np.float32) * dw[p][:, None]
        WqvT.append(np.concatenate([wq.T, wv.T], axis=1))     # [C, 2C]
        WkT.append(np.asarray(Wk[p], np.float32).T)           # [C, C]
        WskipT.append((np.asarray(Wskip[p], np.float32) * dw[p][:, None]).T)
        qv_bias.append(np.concatenate([np.asarray(bq[p], np.float32),
                                       np.asarray(bv[p], np.float32) * dw[p]]))
        k_bias.append(np.asarray(bk[p], np.float32))
        cbias_sum += np.asarray(cbias[p], np.float32) * dw[p]
    cbias_sum += np.asarray(hop_bias, np.float32)

    WqvT = np.stack(WqvT).astype(BF16)        # [P, C, 2C]
    WkT = np.stack(WkT).astype(BF16)          # [P, C, C]
    WskipT = np.stack(WskipT).astype(np.float32)
    qv_bias = np.stack(qv_bias).astype(np.float32)            # [P, 2C]
    k_bias = np.stack(k_bias).astype(np.float32)              # [P, C]

    xT_bf = np.ascontiguousarray(x.T).astype(BF16)            # [C, N]

    # per-hop norm coefficients
    rows, cols, norms = [], [], []
    for p in range(P):
        ei = np.asarray(eis[p])
        row, col = ei[0].astype(np.int64), ei[1].astype(np.int64)
        deg = np.bincount(col, minlength=N).astype(np.float32)
        dinv = np.where(deg > 0, deg ** -0.5, 0.0).astype(np.float32)
        rows.append(row)
        cols.append(col)
        norms.append(dinv[row] * dinv[col] * np.asarray(ews[p], np.float32))

    # node -> (core, bin, pos): LPT pack each shard's nodes into NB bins of
    # <=128, balancing total (3-hop) degree so per-bin edge counts are even.
    deg_tot = np.zeros(N, np.int64)
    for p in range(P):
        deg_tot += np.bincount(cols[p], minlength=N)

    import heapq
    bin_of = np.empty(N, np.int32)
    pos_of = np.empty(N, np.int32)
    perms = []          # per core: node id at each padded slot (or -1)
    for m in range(M):
        lo, hi = m * NS, (m + 1) * NS
        nodes = np.arange(lo, hi)
        order = nodes[np.argsort(-deg_tot[lo:hi], kind="stable")]
        heap = [(0, b, 0) for b in range(NB)]   # (load, bin, count)
        heapq.heapify(heap)
        bins = [[] for _ in range(NB)]
        for nd in order:
            load, b, cnt = heapq.heappop(heap)
            bins[b].append(nd)
            cnt += 1
            load += int(deg_tot[nd])
            if cnt < 128:
                heapq.heappush(heap, (load, b, cnt))
        perm = np.full(NPAD, -1, np.int64)
        for b in range(NB):
            for i, nd in enumerate(bins[b]):
                bin_of[nd] = b
                pos_of[nd] = i
                perm[b * 128 + i] = nd
        perms.append(perm)

    # bucket edges by (core, bin, lo/hi) per hop, then find global tile counts
    per = {}  # (p, m) -> dict of per-bin slot data
    max_lo = max_hi = 1
    for p in range(P):
        row, col, norm = rows[p], cols[p], norms[p]
        core_e = col // NS
        for m in range(M):
            sel = np.nonzero(core_e == m)[0]
            r, c, w = row[sel], col[sel], norm[sel]
            b = bin_of[c]
            hi_side = (r >= SPLIT).astype(np.int64)
            key = b * 2 + hi_side
            ordk = np.argsort(key, kind="stable")
            r, c, w, key = r[ordk], c[ordk], w[ordk], key[ordk]
            cnts = np.bincount(key, minlength=NB * 2)
            offs = np.concatenate([[0], np.cumsum(cnts)])
            per[(p, m)] = (r, c, w, cnts, offs)
            lo_c = cnts[0::2].max() if len(cnts) else 0
            hi_c = cnts[1::2].max() if len(cnts) else 0
            max_lo = max(max_lo, int(lo_c))
            max_hi = max(max_hi, int(hi_c))

    T_LO = (max_lo + 127) // 128
    T_HI = (max_hi + 127) // 128
    T = T_LO + T_HI

    # build per-core input arrays
    in_maps = []
    for m in range(M):
        qv_lo_idx = np.zeros((P, NB, 128, T_LO * 8), np.int16)
        qv_hi_idx = np.zeros((P, NB, 128, T_HI * 8), np.int16)
        kid_idx = np.zeros((P, NB, 128, T * 8), np.int16)
        normv = np.zeros((128, P * NB * T), np.float32)
        for p in range(P):
            r, c, w, cnts, offs = per[(p, m)]
            for b in range(NB):
                nlo = cnts[2 * b]
                nhi = cnts[2 * b + 1]
                olo, ohi = offs[2 * b], offs[2 * b + 1]
                slots_r = np.zeros(T * 128, np.int64)
                slots_cl = np.zeros(T * 128, np.int64)
                slots_w = np.zeros(T * 128, np.float32)
                slots_r[:nlo] = r[olo:olo + nlo]
                slots_cl[:nlo] = pos_of[c[olo:olo + nlo]]
                slots_w[:nlo] = w[olo:olo + nlo]
                hb = T_LO * 128
                slots_r[hb:hb + nhi] = r[ohi:ohi + nhi] - SPLIT
                slots_cl[hb:hb + nhi] = pos_of[c[ohi:ohi + nhi]]
                slots_w[hb:hb + nhi] = w[ohi:ohi + nhi]
                qv_lo_idx[p, b] = _wrap_idx(slots_r[:hb])
                qv_hi_idx[p, b] = _wrap_idx(slots_r[hb:])
                kid_idx[p, b] = _wrap_idx(b * 128 + slots_cl)
                normv[:, (p * NB + b) * T:(p * NB + b + 1) * T] = \
                    slots_w.reshape(T, 128).T
        perm = perms[m]
        xt_perm_bf = np.zeros((128, NPAD), BF16)
        xt_perm_f32 = np.zeros((128, NPAD), np.float32)
        valid = perm >= 0
        xt_perm_bf[:, valid] = xT_bf[:, perm[valid]]
        xt_perm_f32[:, valid] = x.T[:, perm[valid]].astype(np.float32)

        in_maps.append(dict(
            xT=xT_bf,
            xT_perm_bf=xt_perm_bf,
            xT_perm_f32=xt_perm_f32,
            WqvT=np.concatenate(list(WqvT), axis=1),      # [128, P*256]
            WkT=np.concatenate(list(WkT), axis=1),        # [128, P*128]
            WskipT=np.concatenate(list(WskipT), axis=1),  # [128, P*128]
            qv_bias=np.concatenate(
                [np.broadcast_to(qv_bias[p][None, :], (128, 256))
                 for p in range(P)], axis=1).astype(np.float32),
            k_bias=np.concatenate(
                [np.broadcast_to(k_bias[p][None, :], (128, 128))
                 for p in range(P)], axis=1).astype(np.float32),
            cbias=np.broadcast_to(cbias_sum[None, :], (128, 128)).copy()
                   .astype(np.float32),
            ident=np.eye(128, dtype=BF16),
            qv_lo_idx=qv_lo_idx.reshape(P * NB * 128, T_LO * 8),
            qv_hi_idx=qv_hi_idx.reshape(P * NB * 128, T_HI * 8),
            kid_idx=kid_idx.reshape(P * NB * 128, T * 8),
            normv=normv,
        ))
    return in_maps, perms, T_LO, T_HI


def _build(T_LO, T_HI, cfg, phases=7, edge_mode="full"):
    # phases bitmask: 1=node qv, 2=node kid, 4=edge phase
    N, C, P = cfg["N"], cfg["C"], cfg["P"]
    NB, SPLIT, SLAB = cfg["NB"], cfg["SPLIT"], cfg["SLAB"]
    NPAD = NB * 128
    T = T_LO + T_HI
    N_HI = N - SPLIT
    bf = mybir.dt.bfloat16
    f32 = mybir.dt.float32
    i16 = mybir.dt.int16

    nc = bacc.Bacc("TRN2", target_bir_lowering=False, debug=False)

    xT = nc.dram_tensor("xT", [C, N], bf, kind="ExternalInput")
    xT_perm_bf = nc.dram_tensor("xT_perm_bf", [C, NPAD], bf, kind="ExternalInput")
    xT_perm_f32 = nc.dram_tensor("xT_perm_f32", [C, NPAD], f32, kind="ExternalInput")
    WqvT_d = nc.dram_tensor("WqvT", [128, P * 256], bf, kind="ExternalInput")
    WkT_d = nc.dram_tensor("WkT", [128, P * 128], bf, kind="ExternalInput")
    WskipT_d = nc.dram_tensor("WskipT", [128, P * 128], f32, kind="ExternalInput")
    qv_bias_d = nc.dram_tensor("qv_bias", [128, P * 256], f32, kind="ExternalInput")
    k_bias_d = nc.dram_tensor("k_bias", [128, P * 128], f32, kind="ExternalInput")
    cbias_d = nc.dram_tensor("cbias", [128, 128], f32, kind="ExternalInput")
    ident_d = nc.dram_tensor("ident", [128, 128], bf, kind="ExternalInput")
    qv_lo_idx = nc.dram_tensor("qv_lo_idx", [P * NB * 128, T_LO * 8], i16,
                               kind="ExternalInput")
    qv_hi_idx = nc.dram_tensor("qv_hi_idx", [P * NB * 128, T_HI * 8], i16,
                               kind="ExternalInput")
    kid_idx = nc.dram_tensor("kid_idx", [P * NB * 128, T * 8], i16,
                             kind="ExternalInput")
    normv_d = nc.dram_tensor("normv", [128, P * NB * T], f32, kind="ExternalInput")

    out_d = nc.dram_tensor("out", [NPAD, 128], f32, kind="ExternalOutput")

    nblk = (N + 127) // 128

    with TileContext(nc) as tc:
        with tc.tile_pool(name="const", bufs=1) as cp, \
             tc.tile_pool(name="dram", bufs=1, space="DRAM") as dp, \
             tc.tile_pool(name="slab", bufs=2) as slabp, \
             tc.tile_pool(name="qvev", bufs=4) as qvevp, \
             tc.tile_pool(name="kev", bufs=3) as kevp, \
             tc.tile_pool(name="idx", bufs=3) as idxp, \
             tc.tile_pool(name="gath", bufs=2) as gathp, \
             tc.tile_pool(name="ew", bufs=2) as ewp, \
             tc.tile_pool(name="outp", bufs=3) as outp, \
             tc.tile_pool(name="ps_qv", bufs=3, space="PSUM") as ps_qv, \
             tc.tile_pool(name="ps_k", bufs=2, space="PSUM") as ps_k, \
             tc.tile_pool(name="ps_acc", bufs=3, space="PSUM") as ps_acc:

            # resident constants
            wqv_t = cp.tile([128, P * 256], bf)
            nc.sync.dma_start(wqv_t[:], WqvT_d[:, :])
            wk_t = cp.tile([128, P * 128], bf)
            nc.sync.dma_start(wk_t[:], WkT_d[:, :])
            wskip_t = cp.tile([128, P * 128], f32)
            nc.sync.dma_start(wskip_t[:], WskipT_d[:, :])
            qvb_t = cp.tile([128, P * 256], f32)
            nc.sync.dma_start(qvb_t[:], qv_bias_d[:, :])
            kb_t = cp.tile([128, P * 128], f32)
            nc.sync.dma_start(kb_t[:], k_bias_d[:, :])
            cb_t = cp.tile([128, 128], f32)
            nc.sync.dma_start(cb_t[:], cbias_d[:, :])
            id_t = cp.tile([128, 128], bf)
            nc.sync.dma_start(id_t[:], ident_d[:, :])
            xpb_t = cp.tile([C, NPAD], bf)
            nc.sync.dma_start(xpb_t[:], xT_perm_bf[:, :])
            xpf_t = cp.tile([C, NPAD], f32)
            nc.sync.dma_start(xpf_t[:], xT_perm_f32[:, :])
            nrm_t = cp.tile([128, P * NB * T], f32)
            nc.sync.dma_start(nrm_t[:], normv_d[:, :])

            # internal DRAM q|v and k|onehot tables per hop
            qv_lo_t = [dp.tile([SPLIT, 256], bf, name=f"qvlo{p}") for p in range(P)]
            qv_hi_t = [dp.tile([N_HI, 256], bf, name=f"qvhi{p}") for p in range(P)]
            kid_t = [dp.tile([NPAD, 256], bf, name=f"kid{p}") for p in range(P)]

            # ---- Phase A: q|v tables (full N, all hops per xT slab) ----
            for s0 in (range(0, N, SLAB) if phases & 1 else []):
                sw = min(SLAB, N - s0)
                slab = slabp.tile([C, SLAB], bf)
                nc.sync.dma_start(slab[:, :sw], xT[:, s0:s0 + sw])
                for p in range(P):
                    for b0 in range(0, sw, 128):
                        bw = min(128, sw - b0)
                        pq = ps_qv.tile([128, 256], f32)
                        nc.tensor.matmul(pq[:bw], slab[:, b0:b0 + bw],
                                         wqv_t[:, p * 256:(p + 1) * 256],
                                         start=True, stop=True)
                        ev = qvevp.tile([128, 256], bf)
                        nc.any.tensor_tensor(ev[:bw], pq[:bw],
                                             qvb_t[:bw, p * 256:(p + 1) * 256],
                                             mybir.AluOpType.add)
                        g0 = s0 + b0
                        if g0 < SPLIT:
                            nc.sync.dma_start(qv_lo_t[p][g0:g0 + bw, :], ev[:bw])
                        else:
                            nc.sync.dma_start(
                                qv_hi_t[p][g0 - SPLIT:g0 - SPLIT + bw, :], ev[:bw])

            # ---- Phase A2: k|identity tables (shard, permuted order) ----
            for p in (range(P) if phases & 2 else []):
                for b in range(NB):
                    pk = ps_k.tile([128, 128], f32)
                    nc.tensor.matmul(pk[:], xpb_t[:, b * 128:(b + 1) * 128],
                                     wk_t[:, p * 128:(p + 1) * 128],
                                     start=True, stop=True)
                    kev = kevp.tile([128, 256], bf)
                    nc.any.tensor_tensor(kev[:, 0:128], pk[:],
                                         kb_t[:, p * 128:(p + 1) * 128],
                                         mybir.AluOpType.add)
                    nc.any.tensor_copy(kev[:, 128:256], id_t[:])
                    nc.sync.dma_start(kid_t[p][b * 128:(b + 1) * 128, :], kev[:])

            # ---- Phase B: edge processing per bin ----
            for b in (range(NB) if phases & 4 else []):
                acc = ps_acc.tile([128, 128], f32, name="acc") if not edge_mode.startswith("g") else None
                for p in range(P):
                    rb = (p * NB + b) * 128
                    ilo = idxp.tile([128, T_LO * 8], i16, tag="ilo")
                    nc.sync.dma_start(ilo[:], qv_lo_idx[rb:rb + 128, :])
                    ihi = idxp.tile([128, T_HI * 8], i16, tag="ihi")
                    nc.sync.dma_start(ihi[:], qv_hi_idx[rb:rb + 128, :])
                    iki = idxp.tile([128, T * 8], i16, tag="iki")
                    nc.sync.dma_start(iki[:], kid_idx[rb:rb + 128, :])

                    qvg = gathp.tile([128, T, 256], bf, tag="qvg")
                    kg = gathp.tile([128, T, 256], bf, tag="kg")
                    GMAX = 8  # dma_gather is only reliable up to 1024 idxs

                    def gather_pieces(dst, t0, ntiles, src_ap, idx_tile):
                        g0 = 0
                        while g0 < ntiles:
                            gw = min(GMAX, ntiles - g0)
                            nc.gpsimd.dma_gather(
                                dst[:, t0 + g0:t0 + g0 + gw, :], src_ap,
                                idx_tile[:, g0 * 8:(g0 + gw) * 8],
                                gw * 128, gw * 128, 256)
                            g0 += gw

                    do_qv = edge_mode in ("full", "gatheronly", "g_qv")
                    do_kid = edge_mode in ("full", "gatheronly", "g_kid")
                    if not do_qv:
                        nc.vector.memset(qvg[:], 0.25)
                    else:
                        gather_pieces(qvg, 0, T_LO, qv_lo_t[p][:, :], ilo)
                        gather_pieces(qvg, T_LO, T_HI, qv_hi_t[p][:, :], ihi)
                    if not do_kid:
                        nc.vector.memset(kg[:], 0.25)
                    else:
                        gather_pieces(kg, 0, T, kid_t[p][:, :], iki)
                    if edge_mode.startswith("g"):
                        if p == P - 1:
                            got = outp.tile([128, 128], f32, tag="got")
                            nc.vector.tensor_tensor(got[:], kg[:, 0, 0:128],
                                                    qvg[:, 0, 0:128],
                                                    mybir.AluOpType.add)
                            nc.sync.dma_start(out_d[b * 128:(b + 1) * 128, :], got[:])
                        continue

                    s = ewp.tile([128, T, 128], bf, tag="s")
                    nc.vector.tensor_tensor(s[:], kg[:, :, 0:128],
                                            qvg[:, :, 0:128], mybir.AluOpType.add)
                    sg = ewp.tile([128, T, 128], bf, tag="sg")
                    nc.scalar.activation(sg[:], s[:],
                                         mybir.ActivationFunctionType.Sigmoid)
                    mv = ewp.tile([128, T, 128], bf, tag="mv")
                    nc.vector.tensor_tensor(mv[:], sg[:], qvg[:, :, 128:256],
                                            mybir.AluOpType.mult)
                    msg = ewp.tile([128, T, 128], bf, tag="msg")
                    c0 = (p * NB + b) * T
                    nb_ap = nrm_t[:, c0:c0 + T].unsqueeze(2).broadcast_to(
                        [128, T, 128])
                    nc.vector.tensor_tensor(msg[:], mv[:], nb_ap,
                                            mybir.AluOpType.mult)

                    for t in range(T):
                        nc.tensor.matmul(acc[:], kg[:, t, 128:256], msg[:, t, :],
                                         start=(p == 0 and t == 0), stop=False)
                    nc.tensor.matmul(acc[:], xpf_t[:, b * 128:(b + 1) * 128],
                                     wskip_t[:, p * 128:(p + 1) * 128],
                                     start=False, stop=(p == P - 1))

                if not edge_mode.startswith("g"):
                    ot = outp.tile([128, 128], f32)
                    nc.any.tensor_tensor(ot[:], acc[:], cb_t[:], mybir.AluOpType.add)
                    nc.sync.dma_start(out_d[b * 128:(b + 1) * 128, :], ot[:])

    nc.compile()
    return nc


_BUILD_CACHE = {}


def kernel(x, ei1, ei2, ei3, ew1, ew2, ew3,
           Wk, bk, Wq, bq, Wv, bv, Wskip, cbias, d, hop_bias,
           _cfg=None, _want_trace=None):
    cfg = dict(CFG)
    if _cfg:
        cfg.update(_cfg)
    in_maps, perms, T_LO, T_HI = _preprocess(
        x, (ei1, ei2, ei3), (ew1, ew2, ew3),
        Wk, bk, Wq, bq, Wv, bv, Wskip, cbias, d, hop_bias, cfg)

    key = (T_LO, T_HI, tuple(sorted(cfg.items())))
    if key not in _BUILD_CACHE:
        _BUILD_CACHE[key] = _build(T_LO, T_HI, cfg)
    nc = _BUILD_CACHE[key]

    trace = (os.environ.get("BASS_KERNEL_TRACE") == "1"
             if _want_trace is None else _want_trace)
    res = bass_utils.run_bass_kernel_spmd(
        nc, in_maps, core_ids=list(range(cfg["NCORES"])), trace=trace)
    LAST_EXEC_NS[0] = res.exec_time_ns

    N, C = cfg["N"], cfg["C"]
    out = np.zeros((N, C), np.float32)
    for m in range(cfg["NCORES"]):
        o = np.asarray(res.results[m]["out"], np.float32)
        perm = perms[m]
        valid = perm >= 0
        out[perm[valid]] = o[valid]
    return out



# revision 5
# speedup vs baseline: 1.7805x; 1.7805x over previous
"""DMGatedGCNConv (3-hop gated GCN) on 8 Trainium2 NeuronCores.

Strategy (v2): shard target nodes across 8 cores; each core computes the full
q|v projection tables (replicated compute, no collectives) and gathers
per-edge q/v rows with SWDGE dma_gather (the only per-edge gather left).
The k-side is never gathered: per-shard k tables stay SBUF-resident and are
expanded per edge-slot with TensorE one-hot matmuls, using host-shipped
one-hot matrices (graph structure is static). The scatter-reduce also runs
on TensorE with a norm-weighted one-hot, so the edge-norm multiply is free.
Skip connection + biases are folded in via fp32 matmuls and host-side
weight preprocessing.
"""
import os
import sys

sys.path.insert(0, "/opt/trn_rl_repo")

import numpy as np
import ml_dtypes

from concourse import bacc, mybir
from concourse.tile import TileContext
from concourse import bass_utils

BF16 = ml_dtypes.bfloat16

# problem geometry (hardcoded per harness contract)
CFG = dict(
    N=50000,   # nodes
    C=128,     # channels
    P=3,       # hops
    NCORES=8,
    NB=49,     # bins (chunks of <=128 target nodes) per core
    SLAB=2048,  # node-phase xT slab width
    GMAX=8,     # max tiles (128 idx each) per dma_gather
)

LAST_EXEC_NS = [None]  # test harness reads this after kernel()


def _wrap_idx(ids):
    """int16 index array -> [128, L/16] layout dma_gather expects
    (slot i lives at [i % 16, i // 16], replicated for the 8 Q7 cores)."""
    ids = np.asarray(ids, np.int16)
    assert len(ids) % 16 == 0
    return np.tile(np.ascontiguousarray(ids.reshape(-1, 16).T), (8, 1))


def _softmax(d, axis=0):
    e = np.exp(d - d.max(axis=axis, keepdims=True))
    return e / e.sum(axis=axis, keepdims=True)


def _pick_split(rows, cols, bin_of, NB, N, M, NS):
    """Choose the lo/hi table split (int16 gather indices must stay <32768).
    Prefer T_LO=8 (one fewer gather instruction per bin) when the hi side
    stays within 10 tiles; otherwise balance both sides."""
    P = len(rows)
    # flatten every edge to a (global bucket, source row) pair once
    buckets, srcs = [], []
    for p in range(P):
        gb = ((cols[p] // NS) * NB + bin_of[cols[p]]) + p * M * NB
        buckets.append(gb)
        srcs.append(rows[p])
    buckets = np.concatenate(buckets)
    srcs = np.concatenate(srcs)
    nbk = P * M * NB

    def side_tiles(split):
        hi = srcs >= split
        cl = np.bincount(buckets[~hi], minlength=nbk).max() if (~hi).any() else 0
        ch = np.bincount(buckets[hi], minlength=nbk).max() if hi.any() else 0
        return -(-int(cl) // 128), -(-int(ch) // 128)

    # splits stay 128-aligned: the node-phase table writes are 128-row chunks
    lo_min = -(-max(N - 32767, 128) // 128)
    hi_max = 32767 // 128
    # binary search: largest split with max_lo <= 1024 (T_LO == 8)
    a, b = lo_min, min(hi_max, (N - 1) // 128)
    best = None
    while a <= b:
        mid = (a + b) // 2
        tl, th = side_tiles(mid * 128)
        if tl <= 8:
            best = (mid * 128, tl, th)
            a = mid + 1
        else:
            b = mid - 1
    if best is not None and best[1] + best[2] <= 18 and best[2] <= 10:
        return best
    split = min(max(N // 256, lo_min), hi_max) * 128
    tl, th = side_tiles(split)
    return split, tl, th


def _preprocess(x, eis, ews, Wk, bk, Wq, bq, Wv, bv, Wskip, cbias, d, hop_bias, cfg):
    N, C, P = cfg["N"], cfg["C"], cfg["P"]
    M, NB = cfg["NCORES"], cfg["NB"]
    NS = N // M            # nodes per core shard
    NPAD = NB * 128        # padded shard size

    x = np.asarray(x, np.float32)
    dw = _softmax(np.asarray(d, np.float32), axis=0)          # [P, C]

    # fold d_weight into v / skip / cbias; fold hop_bias once
    WqvT, WkT, WskipT = [], [], []
    qv_bias, k_bias = [], []
    cbias_sum = np.zeros(C, np.float32)
    for p in range(P):
        wq = np.asarray(Wq[p], np.float32)
        wv = np.asarray(Wv[p], np.float32) * dw[p][:, None]
        WqvT.append(np.concatenate([wq.T, wv.T], axis=1))     # [C, 2C]
        WkT.append(np.asarray(Wk[p], np.float32).T)           # [C, C]
        WskipT.append((np.asarray(Wskip[p], np.float32) * dw[p][:, None]).T)
        qv_bias.append(np.concatenate([np.asarray(bq[p], np.float32),
                                       np.asarray(bv[p], np.float32) * dw[p]]))
        k_bias.append(np.asarray(bk[p], np.float32))
        cbias_sum += np.asarray(cbias[p], np.float32) * dw[p]
    cbias_sum += np.asarray(hop_bias, np.float32)

    WqvT = np.stack(WqvT).astype(BF16)        # [P, C, 2C]
    WkT = np.stack(WkT).astype(BF16)          # [P, C, C]
    WskipT = np.stack(WskipT).astype(np.float32)
    qv_bias = np.stack(qv_bias).astype(np.float32)            # [P, 2C]
    k_bias = np.stack(k_bias).astype(np.float32)              # [P, C]

    xT_bf = np.ascontiguousarray(x.T).astype(BF16)            # [C, N]

    # per-hop norm coefficients
    rows, cols, norms = [], [], []
    for p in range(P):
        ei = np.asarray(eis[p])
        row, col = ei[0].astype(np.int64), ei[1].astype(np.int64)
        deg = np.bincount(col, minlength=N).astype(np.float32)
        dinv = np.where(deg > 0, deg ** -0.5, 0.0).astype(np.float32)
        rows.append(row)
        cols.append(col)
        norms.append(dinv[row] * dinv[col] * np.asarray(ews[p], np.float32))

    # node -> (core, bin, pos): LPT pack each shard's nodes into NB bins of
    # <=128, balancing total (3-hop) degree so per-bin edge counts are even.
    deg_tot = np.zeros(N, np.int64)
    for p in range(P):
        deg_tot += np.bincount(cols[p], minlength=N)

    import heapq
    bin_of = np.empty(N, np.int32)
    pos_of = np.empty(N, np.int32)
    perms = []          # per core: node id at each padded slot (or -1)
    for m in range(M):
        lo, hi = m * NS, (m + 1) * NS
        nodes = np.arange(lo, hi)
        order = nodes[np.argsort(-deg_tot[lo:hi], kind="stable")]
        heap = [(0, b, 0) for b in range(NB)]   # (load, bin, count)
        heapq.heapify(heap)
        bins = [[] for _ in range(NB)]
        for nd in order:
            load, b, cnt = heapq.heappop(heap)
            bins[b].append(nd)
            cnt += 1
            load += int(deg_tot[nd])
            if cnt < 128:
                heapq.heappush(heap, (load, b, cnt))
        perm = np.full(NPAD, -1, np.int64)
        for b in range(NB):
            for i, nd in enumerate(bins[b]):
                bin_of[nd] = b
                pos_of[nd] = i
                perm[b * 128 + i] = nd
        perms.append(perm)

    SPLIT, T_LO, T_HI = _pick_split(rows, cols, bin_of, NB, N, M, NS)
    T = T_LO + T_HI

    # bucket edges by (core, bin, lo/hi) per hop
    per = {}  # (p, m) -> (r, c, w, cnts, offs)
    for p in range(P):
        row, col, norm = rows[p], cols[p], norms[p]
        core_e = col // NS
        for m in range(M):
            sel = np.nonzero(core_e == m)[0]
            r, c, w = row[sel], col[sel], norm[sel]
            b = bin_of[c]
            hi_side = (r >= SPLIT).astype(np.int64)
            key = b * 2 + hi_side
            ordk = np.argsort(key, kind="stable")
            r, c, w, key = r[ordk], c[ordk], w[ordk], key[ordk]
            cnts = np.bincount(key, minlength=NB * 2)
            offs = np.concatenate([[0], np.cumsum(cnts)])
            per[(p, m)] = (r, c, w, cnts, offs)
            assert cnts[0::2].max() <= T_LO * 128
            assert cnts[1::2].max() <= T_HI * 128

    # build per-core input arrays
    in_maps = []
    for m in range(M):
        qv_lo_idx = np.zeros((P, NB, 128, T_LO * 8), np.int16)
        qv_hi_idx = np.zeros((P, NB, 128, T_HI * 8), np.int16)
        ohT = np.zeros((128, P * NB * T * 128), BF16)   # [t, slot] 0/1
        ohn = np.zeros((128, P * NB * T * 128), BF16)   # [slot%128, tile*128+t] = norm
        for p in range(P):
            r, c, w, cnts, offs = per[(p, m)]
            for b in range(NB):
                nlo = cnts[2 * b]
                nhi = cnts[2 * b + 1]
                olo, ohi = offs[2 * b], offs[2 * b + 1]
                slots_r = np.zeros(T * 128, np.int64)
                slots_cl = np.zeros(T * 128, np.int64)
                slots_w = np.zeros(T * 128, np.float32)
                valid = np.zeros(T * 128, bool)
                slots_r[:nlo] = r[olo:olo + nlo]
                slots_cl[:nlo] = pos_of[c[olo:olo + nlo]]
                slots_w[:nlo] = w[olo:olo + nlo]
                valid[:nlo] = True
                hb = T_LO * 128
                slots_r[hb:hb + nhi] = r[ohi:ohi + nhi] - SPLIT
                slots_cl[hb:hb + nhi] = pos_of[c[ohi:ohi + nhi]]
                slots_w[hb:hb + nhi] = w[ohi:ohi + nhi]
                valid[hb:hb + nhi] = True
                qv_lo_idx[p, b] = _wrap_idx(slots_r[:hb])
                qv_hi_idx[p, b] = _wrap_idx(slots_r[hb:])
                base = (p * NB + b) * T * 128
                s = np.nonzero(valid)[0]
                ohT[slots_cl[s], base + s] = BF16(1.0)
                ohn[s % 128, base + (s // 128) * 128 + slots_cl[s]] = \
                    slots_w[s].astype(BF16)
        perm = perms[m]
        xt_perm_bf = np.zeros((128, NPAD), BF16)
        xt_perm_f32 = np.zeros((128, NPAD), np.float32)
        vmask = perm >= 0
        xt_perm_bf[:, vmask] = xT_bf[:, perm[vmask]]
        xt_perm_f32[:, vmask] = x.T[:, perm[vmask]].astype(np.float32)

        in_maps.append(dict(
            xT=xT_bf,
            xT_perm_bf=xt_perm_bf,
            xT_perm_f32=xt_perm_f32,
            WqvT=np.concatenate(list(WqvT), axis=1),      # [128, P*256]
            WkT=np.concatenate(list(WkT), axis=1),        # [128, P*128]
            WskipT=np.concatenate(list(WskipT), axis=1),  # [128, P*128]
            qv_bias=np.concatenate(
                [np.broadcast_to(qv_bias[p][None, :], (128, 256))
                 for p in range(P)], axis=1).astype(np.float32),
            k_bias=np.concatenate(
                [np.broadcast_to(k_bias[p][None, :], (128, 128))
                 for p in range(P)], axis=1).astype(np.float32),
            cbias=np.broadcast_to(cbias_sum[None, :], (128, 128)).copy()
                   .astype(np.float32),
            qv_lo_idx=qv_lo_idx.reshape(P * NB * 128, T_LO * 8),
            qv_hi_idx=qv_hi_idx.reshape(P * NB * 128, T_HI * 8),
            ohT=ohT,
            ohn=ohn,
        ))
    return in_maps, perms, SPLIT, T_LO, T_HI


def _build(SPLIT, T_LO, T_HI, cfg, phases=7):
    # phases bitmask: 1=node qv+k, 2=unused, 4=edge phase
    N, C, P = cfg["N"], cfg["C"], cfg["P"]
    NB, SLAB, GMAX = cfg["NB"], cfg["SLAB"], cfg["GMAX"]
    NPAD = NB * 128
    T = T_LO + T_HI
    N_HI = N - SPLIT
    bf = mybir.dt.bfloat16
    f32 = mybir.dt.float32
    i16 = mybir.dt.int16

    nc = bacc.Bacc("TRN2", target_bir_lowering=False, debug=False)

    xT = nc.dram_tensor("xT", [C, N], bf, kind="ExternalInput")
    xT_perm_bf = nc.dram_tensor("xT_perm_bf", [C, NPAD], bf, kind="ExternalInput")
    xT_perm_f32 = nc.dram_tensor("xT_perm_f32", [C, NPAD], f32, kind="ExternalInput")
    WqvT_d = nc.dram_tensor("WqvT", [128, P * 256], bf, kind="ExternalInput")
    WkT_d = nc.dram_tensor("WkT", [128, P * 128], bf, kind="ExternalInput")
    WskipT_d = nc.dram_tensor("WskipT", [128, P * 128], f32, kind="ExternalInput")
    qv_bias_d = nc.dram_tensor("qv_bias", [128, P * 256], f32, kind="ExternalInput")
    k_bias_d = nc.dram_tensor("k_bias", [128, P * 128], f32, kind="ExternalInput")
    cbias_d = nc.dram_tensor("cbias", [128, 128], f32, kind="ExternalInput")
    qv_lo_idx = nc.dram_tensor("qv_lo_idx", [P * NB * 128, T_LO * 8], i16,
                               kind="ExternalInput")
    qv_hi_idx = nc.dram_tensor("qv_hi_idx", [P * NB * 128, T_HI * 8], i16,
                               kind="ExternalInput")
    ohT_d = nc.dram_tensor("ohT", [128, P * NB * T * 128], bf,
                           kind="ExternalInput")
    ohn_d = nc.dram_tensor("ohn", [128, P * NB * T * 128], bf,
                           kind="ExternalInput")

    out_d = nc.dram_tensor("out", [NPAD, 128], f32, kind="ExternalOutput")

    with TileContext(nc) as tc:
        with tc.tile_pool(name="const", bufs=1) as cp, \
             tc.tile_pool(name="dram", bufs=1, space="DRAM") as dp, \
             tc.tile_pool(name="slab", bufs=2) as slabp, \
             tc.tile_pool(name="qvev", bufs=4) as qvevp, \
             tc.tile_pool(name="idx", bufs=3) as idxp, \
             tc.tile_pool(name="gath", bufs=3) as gathp, \
             tc.tile_pool(name="oh", bufs=3) as ohp, \
             tc.tile_pool(name="ew", bufs=4) as ewp, \
             tc.tile_pool(name="outp", bufs=3) as outp, \
             tc.tile_pool(name="ps_qv", bufs=2, space="PSUM") as ps_qv, \
             tc.tile_pool(name="ps_k", bufs=1, space="PSUM") as ps_k, \
             tc.tile_pool(name="ps_ke", bufs=3, space="PSUM") as ps_ke, \
             tc.tile_pool(name="ps_acc", bufs=2, space="PSUM") as ps_acc:

            # resident constants
            wqv_t = cp.tile([128, P * 256], bf)
            nc.sync.dma_start(wqv_t[:], WqvT_d[:, :])
            wk_t = cp.tile([128, P * 128], bf)
            nc.sync.dma_start(wk_t[:], WkT_d[:, :])
            wskip_t = cp.tile([128, P * 128], f32)
            nc.sync.dma_start(wskip_t[:], WskipT_d[:, :])
            qvb_t = cp.tile([128, P * 256], f32)
            nc.sync.dma_start(qvb_t[:], qv_bias_d[:, :])
            kb_t = cp.tile([128, P * 128], f32)
            nc.sync.dma_start(kb_t[:], k_bias_d[:, :])
            cb_t = cp.tile([128, 128], f32)
            nc.sync.dma_start(cb_t[:], cbias_d[:, :])
            xpb_t = cp.tile([C, NPAD], bf)
            nc.sync.dma_start(xpb_t[:], xT_perm_bf[:, :])
            xpf_t = cp.tile([C, NPAD], f32)
            nc.sync.dma_start(xpf_t[:], xT_perm_f32[:, :])

            # resident k tables (all hops) and fp32 hop accumulator
            kres = cp.tile([128, P * NB * 128], bf)
            acc_sb = cp.tile([128, NPAD], f32)

            # internal DRAM q|v tables per hop
            qv_lo_t = [dp.tile([SPLIT, 256], bf, name=f"qvlo{p}") for p in range(P)]
            qv_hi_t = [dp.tile([N_HI, 256], bf, name=f"qvhi{p}") for p in range(P)]

            def gather_pieces(dst, t0, ntiles, src_ap, idx_tile):
                g0 = 0
                while g0 < ntiles:
                    gw = min(GMAX, ntiles - g0)
                    nc.gpsimd.dma_gather(
                        dst[:, t0 + g0:t0 + g0 + gw, :], src_ap,
                        idx_tile[:, g0 * 8:(g0 + gw) * 8],
                        gw * 128, gw * 128, 256)
                    g0 += gw

            for p in range(P):
                # ---- k table for this hop (SBUF resident, permuted order) ----
                for b in (range(NB) if phases & 1 else []):
                    pk = ps_k.tile([128, 128], f32)
                    nc.tensor.matmul(pk[:], xpb_t[:, b * 128:(b + 1) * 128],
                                     wk_t[:, p * 128:(p + 1) * 128],
                                     start=True, stop=True)
                    nc.any.tensor_tensor(kres[:, (p * NB + b) * 128:
                                               (p * NB + b + 1) * 128],
                                         pk[:], kb_t[:, p * 128:(p + 1) * 128],
                                         mybir.AluOpType.add)

                # ---- q|v table for this hop (full N, DRAM) ----
                for s0 in (range(0, N, SLAB) if phases & 1 else []):
                    sw = min(SLAB, N - s0)
                    slab = slabp.tile([C, SLAB], bf)
                    nc.sync.dma_start(slab[:, :sw], xT[:, s0:s0 + sw])
                    for b0 in range(0, sw, 128):
                        bw = min(128, sw - b0)
                        pq = ps_qv.tile([128, 256], f32)
                        nc.tensor.matmul(pq[:bw], slab[:, b0:b0 + bw],
                                         wqv_t[:, p * 256:(p + 1) * 256],
                                         start=True, stop=True)
                        ev = qvevp.tile([128, 256], bf)
                        nc.any.tensor_tensor(ev[:bw], pq[:bw],
                                             qvb_t[:bw, p * 256:(p + 1) * 256],
                                             mybir.AluOpType.add)
                        g0 = s0 + b0
                        if g0 < SPLIT:
                            nc.sync.dma_start(qv_lo_t[p][g0:g0 + bw, :], ev[:bw])
                        else:
                            nc.sync.dma_start(
                                qv_hi_t[p][g0 - SPLIT:g0 - SPLIT + bw, :], ev[:bw])

                # ---- edge phase for this hop ----
                for b in (range(NB) if phases & 4 else []):
                    rb = (p * NB + b) * 128
                    kcol = (p * NB + b) * 128
                    ilo = idxp.tile([128, T_LO * 8], i16, tag="ilo")
                    nc.sync.dma_start(ilo[:], qv_lo_idx[rb:rb + 128, :])
                    ihi = idxp.tile([128, T_HI * 8], i16, tag="ihi")
                    nc.sync.dma_start(ihi[:], qv_hi_idx[rb:rb + 128, :])

                    qvg = gathp.tile([128, T, 256], bf, tag="qvg")
                    gather_pieces(qvg, 0, T_LO, qv_lo_t[p][:, :], ilo)
                    gather_pieces(qvg, T_LO, T_HI, qv_hi_t[p][:, :], ihi)

                    ob = (p * NB + b) * T * 128
                    ohT_t = ohp.tile([128, T * 128], bf, tag="ohT")
                    nc.sync.dma_start(ohT_t[:], ohT_d[:, ob:ob + T * 128])
                    ohn_t = ohp.tile([128, T, 128], bf, tag="ohn")
                    nc.sync.dma_start(
                        ohn_t[:],
                        ohn_d.reshape([128, P * NB * T, 128])[:, (p * NB + b) * T:
                                                              (p * NB + b + 1) * T, :])

                    acc = ps_acc.tile([128, 128], f32)
                    for t in range(T):
                        ke = ps_ke.tile([128, 128], f32)
                        nc.tensor.matmul(ke[:], ohT_t[:, t * 128:(t + 1) * 128],
                                         kres[:, kcol:kcol + 128],
                                         start=True, stop=True)
                        s = ewp.tile([128, 128], bf, tag="s")
                        nc.vector.tensor_tensor(s[:], ke[:], qvg[:, t, 0:128],
                                                mybir.AluOpType.add)
                        sg = ewp.tile([128, 128], bf, tag="sg")
                        nc.scalar.activation(sg[:], s[:],
                                             mybir.ActivationFunctionType.Sigmoid)
                        mv = ewp.tile([128, 128], bf, tag="mv")
                        nc.vector.tensor_tensor(mv[:], sg[:], qvg[:, t, 128:256],
                                                mybir.AluOpType.mult)
                        nc.tensor.matmul(acc[:], ohn_t[:, t, :], mv[:],
                                         start=(t == 0), stop=False)
                    nc.tensor.matmul(acc[:], xpf_t[:, b * 128:(b + 1) * 128],
                                     wskip_t[:, p * 128:(p + 1) * 128],
                                     start=False, stop=True)
                    if p == 0:
                        nc.any.tensor_copy(acc_sb[:, b * 128:(b + 1) * 128], acc[:])
                    else:
                        nc.vector.tensor_tensor(
                            acc_sb[:, b * 128:(b + 1) * 128],
                            acc_sb[:, b * 128:(b + 1) * 128], acc[:],
                            mybir.AluOpType.add)

            # ---- output pass ----
            for b in (range(NB) if phases & 4 else []):
                ot = outp.tile([128, 128], f32)
                nc.any.tensor_tensor(ot[:], acc_sb[:, b * 128:(b + 1) * 128],
                                     cb_t[:], mybir.AluOpType.add)
                nc.sync.dma_start(out_d[b * 128:(b + 1) * 128, :], ot[:])

    nc.compile()
    return nc


_BUILD_CACHE = {}


def kernel(x, ei1, ei2, ei3, ew1, ew2, ew3,
           Wk, bk, Wq, bq, Wv, bv, Wskip, cbias, d, hop_bias,
           _cfg=None, _want_trace=None):
    cfg = dict(CFG)
    if _cfg:
        cfg.update(_cfg)
    in_maps, perms, SPLIT, T_LO, T_HI = _preprocess(
        x, (ei1, ei2, ei3), (ew1, ew2, ew3),
        Wk, bk, Wq, bq, Wv, bv, Wskip, cbias, d, hop_bias, cfg)

    key = (SPLIT, T_LO, T_HI, tuple(sorted(cfg.items())))
    if key not in _BUILD_CACHE:
        _BUILD_CACHE[key] = _build(SPLIT, T_LO, T_HI, cfg)
    nc = _BUILD_CACHE[key]

    trace = (os.environ.get("BASS_KERNEL_TRACE") == "1"
             if _want_trace is None else _want_trace)
    res = bass_utils.run_bass_kernel_spmd(
        nc, in_maps, core_ids=list(range(cfg["NCORES"])), trace=trace)
    LAST_EXEC_NS[0] = res.exec_time_ns

    N, C = cfg["N"], cfg["C"]
    out = np.zeros((N, C), np.float32)
    for m in range(cfg["NCORES"]):
        o = np.asarray(res.results[m]["out"], np.float32)
        perm = perms[m]
        valid = perm >= 0
        out[perm[valid]] = o[valid]
    return out


# revision 10
# speedup vs baseline: 2.2341x; 1.2548x over previous
"""DMGatedGCNConv (3-hop gated GCN) on 8 Trainium2 NeuronCores.

Strategy (v2): shard target nodes across 8 cores; each core computes the full
q|v projection tables (replicated compute, no collectives) and gathers
per-edge q/v rows with SWDGE dma_gather (the only per-edge gather left).
The k-side is never gathered: per-shard k tables stay SBUF-resident and are
expanded per edge-slot with TensorE one-hot matmuls, using host-shipped
one-hot matrices (graph structure is static). The scatter-reduce also runs
on TensorE with a norm-weighted one-hot, so the edge-norm multiply is free.
Skip connection + biases are folded in via fp32 matmuls and host-side
weight preprocessing.
"""
import os
import sys

sys.path.insert(0, "/opt/trn_rl_repo")

import numpy as np
import ml_dtypes

from concourse import bacc, mybir
from concourse.tile import TileContext
from concourse import bass_utils

BF16 = ml_dtypes.bfloat16

# problem geometry (hardcoded per harness contract)
CFG = dict(
    N=50000,   # nodes
    C=128,     # channels
    P=3,       # hops
    NCORES=8,
    NB=49,     # bins (chunks of <=128 target nodes) per core
    SLAB=2048,  # node-phase xT slab width
    GMAX=8,     # max tiles (128 idx each) per dma_gather
)

LAST_EXEC_NS = [None]  # test harness reads this after kernel()


def _wrap_idx(ids):
    """int16 index array -> [128, L/16] layout dma_gather expects
    (slot i lives at [i % 16, i // 16], replicated for the 8 Q7 cores)."""
    ids = np.asarray(ids, np.int16)
    assert len(ids) % 16 == 0
    return np.tile(np.ascontiguousarray(ids.reshape(-1, 16).T), (8, 1))


def _softmax(d, axis=0):
    e = np.exp(d - d.max(axis=axis, keepdims=True))
    return e / e.sum(axis=axis, keepdims=True)


def _pick_split(rows, cols, bin_of, NB, N, M, NS):
    """Choose the lo/hi table split (int16 gather indices must stay <32768).
    Prefer T_LO=8 (one fewer gather instruction per bin) when the hi side
    stays within 10 tiles; otherwise balance both sides."""
    P = len(rows)
    # flatten every edge to a (global bucket, source row) pair once
    buckets, srcs = [], []
    for p in range(P):
        gb = ((cols[p] // NS) * NB + bin_of[cols[p]]) + p * M * NB
        buckets.append(gb)
        srcs.append(rows[p])
    buckets = np.concatenate(buckets)
    srcs = np.concatenate(srcs)
    nbk = P * M * NB

    def side_tiles(split):
        hi = srcs >= split
        cl = np.bincount(buckets[~hi], minlength=nbk).max() if (~hi).any() else 0
        ch = np.bincount(buckets[hi], minlength=nbk).max() if hi.any() else 0
        return -(-int(cl) // 128), -(-int(ch) // 128)

    # splits stay 128-aligned: the node-phase table writes are 128-row chunks
    lo_min = -(-max(N - 32767, 128) // 128)
    hi_max = 32767 // 128
    # binary search: largest split with max_lo <= 1024 (T_LO == 8)
    a, b = lo_min, min(hi_max, (N - 1) // 128)
    best = None
    while a <= b:
        mid = (a + b) // 2
        tl, th = side_tiles(mid * 128)
        if tl <= 8:
            best = (mid * 128, tl, th)
            a = mid + 1
        else:
            b = mid - 1
    if best is not None and best[1] + best[2] <= 18 and best[2] <= 10:
        return best
    split = min(max(N // 256, lo_min), hi_max) * 128
    tl, th = side_tiles(split)
    return split, tl, th


def _preprocess(x, eis, ews, Wk, bk, Wq, bq, Wv, bv, Wskip, cbias, d, hop_bias, cfg):
    N, C, P = cfg["N"], cfg["C"], cfg["P"]
    M, NB = cfg["NCORES"], cfg["NB"]
    NS = N // M            # nodes per core shard
    NPAD = NB * 128        # padded shard size

    x = np.asarray(x, np.float32)
    dw = _softmax(np.asarray(d, np.float32), axis=0)          # [P, C]

    # fold d_weight into v / skip / cbias; fold hop_bias once
    WqvT, WkT, WskipT = [], [], []
    qv_bias, k_bias = [], []
    cbias_sum = np.zeros(C, np.float32)
    for p in range(P):
        wq = np.asarray(Wq[p], np.float32)
        wv = np.asarray(Wv[p], np.float32) * dw[p][:, None]
        WqvT.append(np.concatenate([wq.T, wv.T], axis=1))     # [C, 2C]
        WkT.append(np.asarray(Wk[p], np.float32).T)           # [C, C]
        WskipT.append((np.asarray(Wskip[p], np.float32) * dw[p][:, None]).T)
        qv_bias.append(np.concatenate([np.asarray(bq[p], np.float32),
                                       np.asarray(bv[p], np.float32) * dw[p]]))
        k_bias.append(np.asarray(bk[p], np.float32))
        cbias_sum += np.asarray(cbias[p], np.float32) * dw[p]
    cbias_sum += np.asarray(hop_bias, np.float32)

    WqvT = np.stack(WqvT).astype(BF16)        # [P, C, 2C]
    WkT = np.stack(WkT).astype(BF16)          # [P, C, C]
    WskipT = np.stack(WskipT).astype(np.float32)
    qv_bias = np.stack(qv_bias).astype(np.float32)            # [P, 2C]
    k_bias = np.stack(k_bias).astype(np.float32)              # [P, C]

    xT_bf = np.ascontiguousarray(x.T).astype(BF16)            # [C, N]

    # per-hop norm coefficients
    rows, cols, norms = [], [], []
    for p in range(P):
        ei = np.asarray(eis[p])
        row, col = ei[0].astype(np.int64), ei[1].astype(np.int64)
        deg = np.bincount(col, minlength=N).astype(np.float32)
        dinv = np.where(deg > 0, deg ** -0.5, 0.0).astype(np.float32)
        rows.append(row)
        cols.append(col)
        norms.append(dinv[row] * dinv[col] * np.asarray(ews[p], np.float32))

    # node -> (core, bin, pos): LPT pack each shard's nodes into NB bins of
    # <=128, balancing total (3-hop) degree so per-bin edge counts are even.
    deg_tot = np.zeros(N, np.int64)
    for p in range(P):
        deg_tot += np.bincount(cols[p], minlength=N)

    import heapq
    bin_of = np.empty(N, np.int32)
    pos_of = np.empty(N, np.int32)
    perms = []          # per core: node id at each padded slot (or -1)
    for m in range(M):
        lo, hi = m * NS, (m + 1) * NS
        nodes = np.arange(lo, hi)
        order = nodes[np.argsort(-deg_tot[lo:hi], kind="stable")]
        heap = [(0, b, 0) for b in range(NB)]   # (load, bin, count)
        heapq.heapify(heap)
        bins = [[] for _ in range(NB)]
        for nd in order:
            load, b, cnt = heapq.heappop(heap)
            bins[b].append(nd)
            cnt += 1
            load += int(deg_tot[nd])
            if cnt < 128:
                heapq.heappush(heap, (load, b, cnt))
        perm = np.full(NPAD, -1, np.int64)
        for b in range(NB):
            for i, nd in enumerate(bins[b]):
                bin_of[nd] = b
                pos_of[nd] = i
                perm[b * 128 + i] = nd
        perms.append(perm)

    SPLIT, T_LO, T_HI = _pick_split(rows, cols, bin_of, NB, N, M, NS)
    T = T_LO + T_HI

    # bucket edges by (core, bin, lo/hi) per hop
    per = {}  # (p, m) -> (r, c, w, cnts, offs)
    for p in range(P):
        row, col, norm = rows[p], cols[p], norms[p]
        core_e = col // NS
        for m in range(M):
            sel = np.nonzero(core_e == m)[0]
            r, c, w = row[sel], col[sel], norm[sel]
            b = bin_of[c]
            hi_side = (r >= SPLIT).astype(np.int64)
            key = b * 2 + hi_side
            ordk = np.argsort(key, kind="stable")
            r, c, w, key = r[ordk], c[ordk], w[ordk], key[ordk]
            cnts = np.bincount(key, minlength=NB * 2)
            offs = np.concatenate([[0], np.cumsum(cnts)])
            per[(p, m)] = (r, c, w, cnts, offs)
            assert cnts[0::2].max() <= T_LO * 128
            assert cnts[1::2].max() <= T_HI * 128

    # build per-core input arrays
    in_maps = []
    for m in range(M):
        qv_lo_idx = np.zeros((P, NB, 128, T_LO * 8), np.int16)
        qv_hi_idx = np.zeros((P, NB, 128, T_HI * 8), np.int16)
        ohT = np.zeros((128, P * NB * T * 128), BF16)   # [t, slot] 0/1
        ohn = np.zeros((128, P * NB * T * 128), BF16)   # [slot%128, tile*128+t] = norm
        for p in range(P):
            r, c, w, cnts, offs = per[(p, m)]
            for b in range(NB):
                nlo = cnts[2 * b]
                nhi = cnts[2 * b + 1]
                olo, ohi = offs[2 * b], offs[2 * b + 1]
                slots_r = np.zeros(T * 128, np.int64)
                slots_cl = np.zeros(T * 128, np.int64)
                slots_w = np.zeros(T * 128, np.float32)
                valid = np.zeros(T * 128, bool)
                slots_r[:nlo] = r[olo:olo + nlo]
                slots_cl[:nlo] = pos_of[c[olo:olo + nlo]]
                slots_w[:nlo] = w[olo:olo + nlo]
                valid[:nlo] = True
                hb = T_LO * 128
                slots_r[hb:hb + nhi] = r[ohi:ohi + nhi] - SPLIT
                slots_cl[hb:hb + nhi] = pos_of[c[ohi:ohi + nhi]]
                slots_w[hb:hb + nhi] = w[ohi:ohi + nhi]
                valid[hb:hb + nhi] = True
                qv_lo_idx[p, b] = _wrap_idx(slots_r[:hb])
                qv_hi_idx[p, b] = _wrap_idx(slots_r[hb:])
                base = (p * NB + b) * T * 128
                s = np.nonzero(valid)[0]
                ohT[slots_cl[s], base + s] = BF16(1.0)
                ohn[s % 128, base + (s // 128) * 128 + slots_cl[s]] = \
                    slots_w[s].astype(BF16)
        perm = perms[m]
        xt_perm_bf = np.zeros((128, NPAD), BF16)
        xt_perm_f32 = np.zeros((128, NPAD), np.float32)
        vmask = perm >= 0
        xt_perm_bf[:, vmask] = xT_bf[:, perm[vmask]]
        xt_perm_f32[:, vmask] = x.T[:, perm[vmask]].astype(np.float32)

        in_maps.append(dict(
            xT=xT_bf,
            xT_perm_bf=xt_perm_bf,
            xT_perm_f32=xt_perm_f32,
            WqvT=np.concatenate(list(WqvT), axis=1),      # [128, P*256]
            WkT=np.concatenate(list(WkT), axis=1),        # [128, P*128]
            WskipT=np.concatenate(list(WskipT), axis=1),  # [128, P*128]
            qv_bias=np.concatenate(
                [np.broadcast_to(qv_bias[p][None, :], (128, 256))
                 for p in range(P)], axis=1).astype(np.float32),
            k_bias=np.concatenate(
                [np.broadcast_to(k_bias[p][None, :], (128, 128))
                 for p in range(P)], axis=1).astype(np.float32),
            cbias=np.broadcast_to(cbias_sum[None, :], (128, 128)).copy()
                   .astype(np.float32),
            qv_lo_idx=qv_lo_idx.reshape(P * NB * 128, T_LO * 8),
            qv_hi_idx=qv_hi_idx.reshape(P * NB * 128, T_HI * 8),
            ohT=ohT,
            ohn=ohn,
        ))
    return in_maps, perms, SPLIT, T_LO, T_HI


def _build(SPLIT, T_LO, T_HI, cfg, phases=7):
    # phases bitmask: 1=node qv+k, 2=unused, 4=edge phase
    N, C, P = cfg["N"], cfg["C"], cfg["P"]
    NB, SLAB, GMAX = cfg["NB"], cfg["SLAB"], cfg["GMAX"]
    NPAD = NB * 128
    T = T_LO + T_HI
    N_HI = N - SPLIT
    N_HI_PAD = -(-N_HI // 128) * 128
    bf = mybir.dt.bfloat16
    f32 = mybir.dt.float32
    i16 = mybir.dt.int16

    nc = bacc.Bacc("TRN2", target_bir_lowering=False, debug=False)

    xT = nc.dram_tensor("xT", [C, N], bf, kind="ExternalInput")
    xT_perm_bf = nc.dram_tensor("xT_perm_bf", [C, NPAD], bf, kind="ExternalInput")
    xT_perm_f32 = nc.dram_tensor("xT_perm_f32", [C, NPAD], f32, kind="ExternalInput")
    WqvT_d = nc.dram_tensor("WqvT", [128, P * 256], bf, kind="ExternalInput")
    WkT_d = nc.dram_tensor("WkT", [128, P * 128], bf, kind="ExternalInput")
    WskipT_d = nc.dram_tensor("WskipT", [128, P * 128], f32, kind="ExternalInput")
    qv_bias_d = nc.dram_tensor("qv_bias", [128, P * 256], f32, kind="ExternalInput")
    k_bias_d = nc.dram_tensor("k_bias", [128, P * 128], f32, kind="ExternalInput")
    cbias_d = nc.dram_tensor("cbias", [128, 128], f32, kind="ExternalInput")
    qv_lo_idx = nc.dram_tensor("qv_lo_idx", [P * NB * 128, T_LO * 8], i16,
                               kind="ExternalInput")
    qv_hi_idx = nc.dram_tensor("qv_hi_idx", [P * NB * 128, T_HI * 8], i16,
                               kind="ExternalInput")
    ohT_d = nc.dram_tensor("ohT", [128, P * NB * T * 128], bf,
                           kind="ExternalInput")
    ohn_d = nc.dram_tensor("ohn", [128, P * NB * T * 128], bf,
                           kind="ExternalInput")

    out_d = nc.dram_tensor("out", [NPAD, 128], f32, kind="ExternalOutput")

    with TileContext(nc) as tc:
        with tc.tile_pool(name="const", bufs=1) as cp, \
             tc.tile_pool(name="dram", bufs=1, space="DRAM") as dp, \
             tc.tile_pool(name="slab", bufs=2) as slabp, \
             tc.tile_pool(name="qvev", bufs=4) as qvevp, \
             tc.tile_pool(name="idx", bufs=3) as idxp, \
             tc.tile_pool(name="gath", bufs=3) as gathp, \
             tc.tile_pool(name="oh", bufs=3) as ohp, \
             tc.tile_pool(name="ew", bufs=4) as ewp, \
             tc.tile_pool(name="outp", bufs=3) as outp, \
             tc.tile_pool(name="ps_qv", bufs=2, space="PSUM") as ps_qv, \
             tc.tile_pool(name="ps_k", bufs=1, space="PSUM") as ps_k, \
             tc.tile_pool(name="ps_ke", bufs=3, space="PSUM") as ps_ke, \
             tc.tile_pool(name="ps_acc", bufs=2, space="PSUM") as ps_acc:

            # resident constants
            wqv_t = cp.tile([128, P * 256], bf)
            nc.sync.dma_start(wqv_t[:], WqvT_d[:, :])
            wk_t = cp.tile([128, P * 128], bf)
            nc.sync.dma_start(wk_t[:], WkT_d[:, :])
            wskip_t = cp.tile([128, P * 128], f32)
            nc.sync.dma_start(wskip_t[:], WskipT_d[:, :])
            qvb_t = cp.tile([128, P * 256], f32)
            nc.sync.dma_start(qvb_t[:], qv_bias_d[:, :])
            kb_t = cp.tile([128, P * 128], f32)
            nc.sync.dma_start(kb_t[:], k_bias_d[:, :])
            cb_t = cp.tile([128, 128], f32)
            nc.sync.dma_start(cb_t[:], cbias_d[:, :])
            xpb_t = cp.tile([C, NPAD], bf)
            nc.sync.dma_start(xpb_t[:], xT_perm_bf[:, :])
            xpf_t = cp.tile([C, NPAD], f32)
            nc.sync.dma_start(xpf_t[:], xT_perm_f32[:, :])

            # resident k tables (all hops) and fp32 hop accumulator
            kres = cp.tile([128, P * NB * 128], bf)
            acc_sb = cp.tile([128, NPAD], f32)

            # internal DRAM q|v tables per hop
            qv_lo_t = [dp.tile([SPLIT, 256], bf, name=f"qvlo{p}") for p in range(P)]
            qv_hi_t = [dp.tile([N_HI_PAD, 256], bf, name=f"qvhi{p}") for p in range(P)]

            def gather_pieces(dst, t0, ntiles, src_ap, idx_tile):
                g0 = 0
                while g0 < ntiles:
                    gw = min(GMAX, ntiles - g0)
                    nc.gpsimd.dma_gather(
                        dst[:, t0 + g0:t0 + g0 + gw, :], src_ap,
                        idx_tile[:, g0 * 8:(g0 + gw) * 8],
                        gw * 128, gw * 128, 256)
                    g0 += gw

            def emit_a2_chunk(p, b0, bn):
                # k table rows for bins [b0, b0+bn) of hop p (SBUF resident)
                for b in range(b0, min(b0 + bn, NB)):
                    pk = ps_k.tile([128, 128], f32)
                    nc.tensor.matmul(pk[:], xpb_t[:, b * 128:(b + 1) * 128],
                                     wk_t[:, p * 128:(p + 1) * 128],
                                     start=True, stop=True)
                    nc.any.tensor_tensor(kres[:, (p * NB + b) * 128:
                                               (p * NB + b + 1) * 128],
                                         pk[:], kb_t[:, p * 128:(p + 1) * 128],
                                         mybir.AluOpType.add)

            def emit_a_slab(p, s0):
                # one xT slab -> q|v table rows, written with batched DMAs
                sw = min(SLAB, N - s0)
                nch = (sw + 127) // 128
                full = sw // 128          # full 128-row chunks
                rem = sw - full * 128
                slab = slabp.tile([C, SLAB], bf)
                nc.sync.dma_start(slab[:, :sw], xT[:, s0:s0 + sw])
                sev = qvevp.tile([128, SLAB // 128, 256], bf)
                for j in range(nch):
                    b0 = j * 128
                    bw = min(128, sw - b0)
                    pq = ps_qv.tile([128, 256], f32)
                    nc.tensor.matmul(pq[:bw], slab[:, b0:b0 + bw],
                                     wqv_t[:, p * 256:(p + 1) * 256],
                                     start=True, stop=True)
                    nc.any.tensor_tensor(sev[:bw, j, :], pq[:bw],
                                         qvb_t[:bw, p * 256:(p + 1) * 256],
                                         mybir.AluOpType.add)
                # batched table writes for full chunks (split is 128-aligned)
                lo_t = qv_lo_t[p].tensor.reshape([SPLIT // 128, 128, 256])
                hi_t = qv_hi_t[p].tensor.reshape([N_HI_PAD // 128, 128, 256])
                for side in (0, 1):
                    if side == 0:
                        a, bb = s0, min(s0 + full * 128, SPLIT)
                        if bb <= a:
                            continue
                        dst = lo_t[a // 128:bb // 128]
                    else:
                        a, bb = max(s0, SPLIT), s0 + full * 128
                        if bb <= a:
                            continue
                        dst = hi_t[(a - SPLIT) // 128:(bb - SPLIT) // 128]
                    j0 = (a - s0) // 128
                    k = (bb - a) // 128
                    nc.sync.dma_start(dst.transpose([1, 0, 2]),
                                      sev[:, j0:j0 + k, :])
                if rem:
                    a = s0 + full * 128
                    assert a >= SPLIT, "remainder must land in the hi table"
                    nc.sync.dma_start(qv_hi_t[p][a - SPLIT:a - SPLIT + rem, :],
                                      sev[:rem, full, :])

            n_slab = (N + SLAB - 1) // SLAB
            if phases & 1:
                emit_a2_chunk(0, 0, NB)
                for s0 in range(0, N, SLAB):
                    emit_a_slab(0, s0)

            for p in range(P):
                # next hop's table work, spread over this hop's bin loop
                tasks = []
                if p + 1 < P and phases & 1:
                    tasks += [("a2", p + 1, b0) for b0 in range(0, NB, 10)]
                    tasks += [("a", p + 1, s0) for s0 in range(0, N, SLAB)]
                ti = 0

                # ---- edge phase for this hop ----
                for b in (range(NB) if phases & 4 else []):
                    rb = (p * NB + b) * 128
                    kcol = (p * NB + b) * 128
                    ilo = idxp.tile([128, T_LO * 8], i16, tag="ilo")
                    nc.sync.dma_start(ilo[:], qv_lo_idx[rb:rb + 128, :])
                    ihi = idxp.tile([128, T_HI * 8], i16, tag="ihi")
                    nc.sync.dma_start(ihi[:], qv_hi_idx[rb:rb + 128, :])

                    qvg = gathp.tile([128, T, 256], bf, tag="qvg")
                    gather_pieces(qvg, 0, T_LO, qv_lo_t[p][:, :], ilo)
                    gather_pieces(qvg, T_LO, T_HI, qv_hi_t[p][:, :], ihi)

                    ob = (p * NB + b) * T * 128
                    ohT_t = ohp.tile([128, T * 128], bf, tag="ohT")
                    nc.sync.dma_start(ohT_t[:], ohT_d[:, ob:ob + T * 128])
                    ohn_t = ohp.tile([128, T, 128], bf, tag="ohn")
                    nc.sync.dma_start(
                        ohn_t[:],
                        ohn_d.reshape([128, P * NB * T, 128])[:, (p * NB + b) * T:
                                                              (p * NB + b + 1) * T, :])

                    acc = ps_acc.tile([128, 128], f32)
                    for t in range(T):
                        ke = ps_ke.tile([128, 128], f32)
                        nc.tensor.matmul(ke[:], ohT_t[:, t * 128:(t + 1) * 128],
                                         kres[:, kcol:kcol + 128],
                                         start=True, stop=True)
                        s = ewp.tile([128, 128], bf, tag="s")
                        nc.vector.tensor_tensor(s[:], ke[:], qvg[:, t, 0:128],
                                                mybir.AluOpType.add)
                        sg = ewp.tile([128, 128], bf, tag="sg")
                        nc.scalar.activation(sg[:], s[:],
                                             mybir.ActivationFunctionType.Sigmoid)
                        mv = ewp.tile([128, 128], bf, tag="mv")
                        nc.vector.tensor_tensor(mv[:], sg[:], qvg[:, t, 128:256],
                                                mybir.AluOpType.mult)
                        nc.tensor.matmul(acc[:], ohn_t[:, t, :], mv[:],
                                         start=(t == 0), stop=False)
                    nc.tensor.matmul(acc[:], xpf_t[:, b * 128:(b + 1) * 128],
                                     wskip_t[:, p * 128:(p + 1) * 128],
                                     start=False, stop=True)
                    if p == 0:
                        nc.any.tensor_copy(acc_sb[:, b * 128:(b + 1) * 128], acc[:])
                    else:
                        nc.vector.tensor_tensor(
                            acc_sb[:, b * 128:(b + 1) * 128],
                            acc_sb[:, b * 128:(b + 1) * 128], acc[:],
                            mybir.AluOpType.add)

                    # spread next-hop table building across this hop's bins
                    due = ((b + 1) * len(tasks)) // NB
                    while ti < due:
                        kind, tp, targ = tasks[ti]
                        if kind == "a2":
                            emit_a2_chunk(tp, targ, 10)
                        else:
                            emit_a_slab(tp, targ)
                        ti += 1

            # ---- output pass ----
            for b in (range(NB) if phases & 4 else []):
                ot = outp.tile([128, 128], f32)
                nc.any.tensor_tensor(ot[:], acc_sb[:, b * 128:(b + 1) * 128],
                                     cb_t[:], mybir.AluOpType.add)
                nc.sync.dma_start(out_d[b * 128:(b + 1) * 128, :], ot[:])

    nc.compile()
    return nc


_BUILD_CACHE = {}


def kernel(x, ei1, ei2, ei3, ew1, ew2, ew3,
           Wk, bk, Wq, bq, Wv, bv, Wskip, cbias, d, hop_bias,
           _cfg=None, _want_trace=None):
    cfg = dict(CFG)
    if _cfg:
        cfg.update(_cfg)
    in_maps, perms, SPLIT, T_LO, T_HI = _preprocess(
        x, (ei1, ei2, ei3), (ew1, ew2, ew3),
        Wk, bk, Wq, bq, Wv, bv, Wskip, cbias, d, hop_bias, cfg)

    key = (SPLIT, T_LO, T_HI, tuple(sorted(cfg.items())))
    if key not in _BUILD_CACHE:
        _BUILD_CACHE[key] = _build(SPLIT, T_LO, T_HI, cfg)
    nc = _BUILD_CACHE[key]

    trace = (os.environ.get("BASS_KERNEL_TRACE") == "1"
             if _want_trace is None else _want_trace)
    res = bass_utils.run_bass_kernel_spmd(
        nc, in_maps, core_ids=list(range(cfg["NCORES"])), trace=trace)
    LAST_EXEC_NS[0] = res.exec_time_ns

    N, C = cfg["N"], cfg["C"]
    out = np.zeros((N, C), np.float32)
    for m in range(cfg["NCORES"]):
        o = np.asarray(res.results[m]["out"], np.float32)
        perm = perms[m]
        valid = perm >= 0
        out[perm[valid]] = o[valid]
    return out
